# revision 38
# baseline (speedup 1.0000x reference)
"""Causal self-attention (B=2, T=2048, C=768, H=12) on 8 TRN2 NeuronCores.

Sharding: data-parallel over B (cores 0-3 -> b=0, cores 4-7 -> b=1), tensor
parallel over heads (3 heads per core). Each core computes q/k/v projections
for its 3 heads, causal attention, and a partial output projection; the host
sums the 4 partials per batch element and adds the output/v biases.

Attention is computed transposed: S^T[tk, tq] = K Q^T so that the softmax
denominator comes out of the ones-augmented AV matmul (V | 1) as row 64 of
the [65, 512] PSUM accumulator; exp runs on the scalar engine straight out
of PSUM with 1/sqrt(d) folded into the activation scale.

PE sub-array concurrency (measured 1.95x for K=64 pairs on disjoint row
strips): the S matmuls for heads A and B occupy PE row strips 0-63 / 64-127
and write the two banks of one [128, 1024] PSUM tile, so the pair executes
concurrently and one merged exp covers both heads. Head C's S matmuls are
paired tile-on-tile the same way using a second copy of qC/kC on the other
partition strip. The K=64 half of the out-projection is likewise paired
across jt using duplicated w2c / ot_c rows.

The attention phase is ACT(exp)-paced, so the q/k/v projection matmuls for
chunk j+1 and the out-projection for chunk j-1 are injected between the
attention tile-steps of chunk j to keep PE busy while ACT drains.

The v bias never touches the device: softmax rows sum to 1, so its
contribution is the constant vector out_w @ qkv_b[2C:], added on the host.
"""

import numpy as np
import ml_dtypes
from contextlib import ExitStack

import concourse.bass as bass
import concourse.tile as tile
from concourse import bacc, mybir
from concourse.bass_utils import run_bass_kernel_spmd

BF16 = mybir.dt.bfloat16
F16 = mybir.dt.float16
F32 = mybir.dt.float32
AF = mybir.ActivationFunctionType

B, T, C, H, D = 2, 2048, 768, 12, 64
HPC = 3          # heads per core
NCORES = 8
CC = C // 128    # 6 contraction chunks
NT = T // 128    # 16 t tiles
NJ = T // 512    # 4 tq chunks
VW = D + 1       # 65: v columns + ones column
SCALE = float(D) ** -0.5

_cache = {}


def _build_program():
    nc = bacc.Bacc("TRN2", target_bir_lowering=False, debug=False,
                   enable_asserts=False, num_devices=NCORES)

    # xt: j-major [j, kc, 512]; wqk: jt-major [jt, kc, 128]
    xt_d = nc.dram_tensor("xt_s", [128, NJ * CC * 512], BF16,
                          kind="ExternalInput").ap()
    wqk_d = nc.dram_tensor("wqk_s", [128, 3 * CC * 128], BF16,
                           kind="ExternalInput").ap()
    wv_d = nc.dram_tensor("wv_s", [128, CC * 192], BF16,
                          kind="ExternalInput").ap()
    bqk_d = nc.dram_tensor("bqk_s", [128, 3], F32, kind="ExternalInput").ap()
    w2ab_d = nc.dram_tensor("w2ab_s", [128, C], BF16, kind="ExternalInput").ap()
    w2c_d = nc.dram_tensor("w2c_s", [64, C], BF16, kind="ExternalInput").ap()
    # causal edge mask [128, 128] (same for every diag offset), doubled for
    # the merged A/B diag steps: [mask | mask]
    masks_d = nc.dram_tensor("masks_s", [128, 256], BF16,
                             kind="ExternalInput").ap()
    outp_d = nc.dram_tensor("outp", [128, CC * T], F16, kind="ExternalOutput").ap()

    with tile.TileContext(nc) as tc, ExitStack() as ctx:
        const = ctx.enter_context(tc.tile_pool(name="const", bufs=1))
        big = ctx.enter_context(tc.tile_pool(name="big", bufs=1))
        # PSUM budget (8 banks): s 2x[128,1024] = 4, av 2x[65,512] = 2,
        # pr 2x[128,512] = 2
        psum_s = ctx.enter_context(tc.tile_pool(name="psum_s", bufs=2,
                                                space="PSUM"))
        psum_av = ctx.enter_context(tc.tile_pool(name="psum_av", bufs=2,
                                                 space="PSUM"))
        psum_pr = ctx.enter_context(tc.tile_pool(name="psum_pr", bufs=2,
                                                 space="PSUM"))
        ppool = ctx.enter_context(tc.tile_pool(name="ppool", bufs=4))
        small = ctx.enter_context(tc.tile_pool(name="small", bufs=6))

        # warm the ACT exp table while DMAs are in flight
        warm = small.tile([1, 16], F32, tag="warm")
        nc.vector.memset(warm[:], 0.0)
        warm2 = small.tile([1, 16], F32, tag="warm")
        nc.scalar.activation(warm2[:], warm[:], AF.Exp)

        # ---- load constants/inputs (order matters: earliest consumers first)
        wqk = const.tile([128, 3 * CC * 128], BF16)
        nc.sync.dma_start(wqk[:, 0:768], wqk_d[:, 0:768])       # jt=0
        xt = const.tile([128, NJ * CC * 512], BF16)
        for kc in range(CC):                                    # xt chunk j=0
            sl = slice(kc * 512, (kc + 1) * 512)
            nc.sync.dma_start(xt[:, sl], xt_d[:, sl])
        nc.sync.dma_start(wqk[:, 768:1536], wqk_d[:, 768:1536])   # jt=1
        nc.sync.dma_start(wqk[:, 1536:2304], wqk_d[:, 1536:2304])  # jt=2
        bqk = const.tile([128, 3], F32)
        nc.sync.dma_start(bqk[:], bqk_d[:])
        wv = const.tile([128, CC * 192], BF16)
        nc.sync.dma_start(wv[:], wv_d[:])
        nc.sync.dma_start(xt[:, 3072:6144], xt_d[:, 3072:6144])   # j=1
        masks = const.tile([128, 256], BF16)
        nc.sync.dma_start(masks[:], masks_d[:])
        w2ab = const.tile([128, C], BF16)
        nc.sync.dma_start(w2ab[:], w2ab_d[:])
        w2c = const.tile([128, C], BF16)   # same 64 rows on both strips
        nc.sync.dma_start(w2c[0:64, :], w2c_d[:])
        nc.sync.dma_start(w2c[64:128, :], w2c_d[:])
        nc.sync.dma_start(xt[:, 6144:9216], xt_d[:, 6144:9216])   # j=2
        nc.sync.dma_start(xt[:, 9216:12288], xt_d[:, 9216:12288])  # j=3

        # ---- persistent intermediates
        qt1 = big.tile([128, T], BF16)   # qA (p 0-63) | qB (p 64-127), [d, t]
        kt1 = big.tile([128, T], BF16)   # kA | kB
        qkt2 = big.tile([128, T], BF16)  # qC | kC
        kt2 = big.tile([128, T], BF16)   # kC (p 0-63) | qC (p 64-127)
        vbuf = big.tile([128, NT * HPC * VW], BF16)  # per t-chunk: [vA 1|vB 1|vC 1]
        ot_ab = big.tile([128, T], BF16)  # O.T heads A,B (out-proj rhs chunk 0)
        ot_c = big.tile([128, T], BF16)   # O.T head C on both strips

        nc.gpsimd.memset(vbuf[:], 1.0)

        def v_ap(h, i):
            off = i * HPC * VW + h * VW
            return vbuf[:, off:off + VW]

        qk_dest = [qt1, kt1, qkt2]

        def xt_ap(kc, j):
            return xt[:, j * 3072 + kc * 512: j * 3072 + (kc + 1) * 512]

        def qk_parts(jt, j):
            """Injectable closures: 3x two accumulating matmuls + bias add."""
            state = {}

            def mk(kc0):
                def f():
                    if "ps" not in state:
                        state["ps"] = psum_pr.tile([128, 512], F32, tag="proj",
                                                   name=f"qk_{jt}_{j}")
                    for kc in (kc0, kc0 + 1):
                        nc.tensor.matmul(
                            state["ps"][:],
                            wqk[:, jt * 768 + kc * 128: jt * 768 + (kc + 1) * 128],
                            xt_ap(kc, j),
                            start=(kc == 0), stop=(kc == CC - 1),
                        )
                return f

            def fin():
                nc.vector.tensor_scalar_add(
                    qk_dest[jt][:, j * 512:(j + 1) * 512], state["ps"][:],
                    bqk[:, jt:jt + 1])

            return [mk(0), mk(2), mk(4), fin]

        def emit_qk_group(jt, j):
            for f in qk_parts(jt, j):
                f()

        def emit_shift(j):
            js = slice(j * 512, (j + 1) * 512)
            nc.sync.dma_start(kt2[0:64, js], qkt2[64:128, js])
            nc.sync.dma_start(kt2[64:128, js], qkt2[0:64, js])

        def v_parts(ti):
            state = {}
            jv, rv = ti // 4, ti % 4

            def mk(kc0):
                def f():
                    if "ps" not in state:
                        state["ps"] = psum_pr.tile([128, 192], F32, tag="proj",
                                                   name=f"v_{ti}")
                    for kc in (kc0, kc0 + 1):
                        nc.tensor.matmul(
                            state["ps"][:],
                            xt[:, jv * 3072 + kc * 512 + rv * 128:
                               jv * 3072 + kc * 512 + (rv + 1) * 128],
                            wv[:, kc * 192:(kc + 1) * 192],
                            start=(kc == 0), stop=(kc == CC - 1),
                        )
                return f

            def fin():
                dst = vbuf[:, ti * HPC * VW:(ti + 1) * HPC * VW]
                dst = dst.rearrange("p (h x) -> p h x", h=HPC)[:, :, 0:D]
                nc.vector.tensor_copy(
                    dst, state["ps"][:].rearrange("p (h x) -> p h x", h=HPC))

            return [mk(0), mk(2), mk(4), fin]

        def emit_v_group(ti):
            for f in v_parts(ti):
                f()

        def op_parts(p, j):
            # jt pair (2p, 2p+1); the K=64 w2c matmuls run concurrently on
            # opposite PE row strips.
            jt0, jt1 = 2 * p, 2 * p + 1
            js = slice(j * 512, (j + 1) * 512)
            state = {}

            def mm1():
                ps0 = psum_pr.tile([128, 512], F32, tag="proj",
                                   name=f"op0_{p}_{j}")
                ps1 = psum_pr.tile([128, 512], F32, tag="proj",
                                   name=f"op1_{p}_{j}")
                state["ps"] = (ps0, ps1)
                nc.tensor.matmul(ps0[:], w2ab[:, jt0 * 128:(jt0 + 1) * 128],
                                 ot_ab[:, js], start=True, stop=False,
                                 skip_group_check=True)
                nc.tensor.matmul(ps1[:], w2ab[:, jt1 * 128:(jt1 + 1) * 128],
                                 ot_ab[:, js], start=True, stop=False,
                                 skip_group_check=True)

            def mm2():
                ps0, ps1 = state["ps"]
                nc.tensor.matmul(ps0[:], w2c[0:64, jt0 * 128:(jt0 + 1) * 128],
                                 ot_c[0:64, js], start=False, stop=True,
                                 skip_group_check=True)
                nc.tensor.matmul(ps1[:], w2c[64:128, jt1 * 128:(jt1 + 1) * 128],
                                 ot_c[64:128, js], start=False, stop=True,
                                 skip_group_check=True)

            def fin():
                for jt, ps in zip((jt0, jt1), state["ps"]):
                    ob = small.tile([128, 512], F16, tag="ob",
                                    name=f"ob_{jt}_{j}")
                    nc.vector.tensor_copy(ob[:], ps[:])
                    nc.sync.dma_start(
                        outp_d[:, jt * T + j * 512: jt * T + (j + 1) * 512],
                        ob[:])

            return [mm1, mm2, fin]

        def emit_outproj_pair(p, j):
            for f in op_parts(p, j):
                f()

        # warm-up matmuls: PE activity during the input DMA wait so the HAM
        # clock-gate ramps before real work starts
        wz = const.tile([128, 512], BF16, name="wz")
        nc.gpsimd.memset(wz[:], 0.0)
        for i in range(8):
            wps = psum_pr.tile([128, 512], F32, tag="proj", name=f"wps{i}")
            nc.tensor.matmul(wps[:], wz[:, 0:128], wz[:], start=True, stop=True)

        # prologue: only the projections gating chunk 0's first S-pair;
        # the rest goes on the deadline queue below
        emit_qk_group(0, 0)
        emit_qk_group(1, 0)

        # Global injection queue: ~2-matmul closures popped ONE at a time at
        # pump points inside attention tile-steps, so the next S-pair is never
        # buried behind a convoy of projection matmuls (which starves ACT).
        # Items carry a deadline; force_flush runs everything due by a phase.
        inj_q = []   # (deadline | None, closure)
        inj_q.extend((("vd", 0), f) for ti in range(4) for f in v_parts(ti))
        inj_q.extend((("c", 0), f) for f in qk_parts(2, 0))
        inj_q.append((("c", 0), lambda: emit_shift(0)))

        def pump(n=1):
            for _ in range(min(n, len(inj_q))):
                inj_q.pop(0)[1]()

        def force_flush(dl):
            keep = []
            for item in inj_q:
                if item[0] == dl:
                    item[1]()
                else:
                    keep.append(item)
            inj_q[:] = keep

        for j in range(NJ):
            js = slice(j * 512, (j + 1) * 512)

            if j + 1 < NJ:
                for jt in (0, 1):
                    inj_q.extend((("ab", j + 1), f) for f in qk_parts(jt, j + 1))
                inj_q.extend((("c", j + 1), f) for f in qk_parts(2, j + 1))
                inj_q.append((("c", j + 1), lambda jn=j + 1: emit_shift(jn)))
                for ti in range(4 * (j + 1), 4 * (j + 1) + 4):
                    inj_q.extend((("vd", j + 1), f) for f in v_parts(ti))
            if j - 1 >= 0:
                for p in range(3):
                    inj_q.extend((None, f) for f in op_parts(p, j - 1))

            # everything chunk j's AB group depends on must be emitted already
            force_flush(("ab", j))

            def maybe_inject():
                pump()

            # ---------------- group A,B (row strips 0-63 / 64-127) --------
            av = {h: psum_av.tile([VW, 512], F32, tag="av",
                                  name=f"av_{h}_{j}") for h in (0, 1)}
            started = {0: False, 1: False}
            prev = []   # (h, i, pt_ap, col0) from previous tile-step

            def flush_av(last=False):
                for (h, i, pt_ap, c0) in prev:
                    nc.tensor.matmul(
                        av[h][:, c0:512], v_ap(h, i), pt_ap,
                        start=(not started[h]), stop=last,
                        skip_group_check=True,
                    )
                    started[h] = True
                prev.clear()

            # full tiles (i < 4j): S_A and S_B write the two banks of one
            # [128, 1024] PSUM tile and execute concurrently; one merged exp.
            for i0 in range(4 * j):
                it = slice(i0 * 128, (i0 + 1) * 128)
                st = psum_s.tile([128, 1024], F32, tag="s",
                                 name=f"sab_{j}_{i0}")
                nc.tensor.matmul(st[:, 0:512], kt1[0:64, it], qt1[0:64, js],
                                 start=True, stop=True, skip_group_check=True)
                nc.tensor.matmul(st[:, 512:1024], kt1[64:128, it],
                                 qt1[64:128, js],
                                 start=True, stop=True, skip_group_check=True)
                pt = ppool.tile([128, 1024], BF16, tag="pt",
                                name=f"ptab_{j}_{i0}")
                nc.scalar.activation(pt[:], st[:], AF.Exp, scale=SCALE)
                pump()
                flush_av()
                prev.extend([(0, i0, pt[:, 0:512], 0),
                             (1, i0, pt[:, 512:1024], 0)])
                maybe_inject()

            # diagonal tiles i = 4j+oi, restricted to cols >= 128*oi
            force_flush(("vd", j))
            for oi in range(4):
                i = 4 * j + oi
                it = slice(i * 128, (i + 1) * 128)
                c0 = 128 * oi
                w = 512 - c0
                qs = slice(j * 512 + c0, (j + 1) * 512)
                st = psum_s.tile([128, 1024], F32, tag="s",
                                 name=f"sabd_{j}_{oi}")
                nc.tensor.matmul(st[:, c0:512], kt1[0:64, it], qt1[0:64, qs],
                                 start=True, stop=True, skip_group_check=True)
                nc.tensor.matmul(st[:, 512 + c0:1024], kt1[64:128, it],
                                 qt1[64:128, qs],
                                 start=True, stop=True, skip_group_check=True)
                pt = ppool.tile([128, 1024], BF16, tag="pt",
                                name=f"ptabd_{j}_{oi}")
                st3 = st[:].rearrange("p (two q) -> p two q", two=2)[:, :, c0:512]
                pt3 = pt[:].rearrange("p (two q) -> p two q", two=2)[:, :, c0:512]
                nc.scalar.activation(pt3, st3, AF.Exp, scale=SCALE)
                # the invalid triangle only spans cols [c0, c0+128)
                ptm = pt[:].rearrange("p (two q) -> p two q", two=2) \
                    [:, :, c0:c0 + 128]
                m3 = masks[:].rearrange("p (two q) -> p two q", two=2)
                nc.vector.tensor_mul(ptm, ptm, m3)
                pump(2)
                flush_av()
                prev.extend([(0, i, pt[:, c0:512], c0),
                             (1, i, pt[:, 512 + c0:1024], c0)])
                maybe_inject()
            flush_av(last=True)

            # normalize A, B: O.T[d, tq] = av[0:64] / av[64].  The whole av
            # accumulator is copied to SBUF first so the PSUM bank frees in
            # one op instead of being held through the normalize chain; the
            # copy doubles as the den staging (custom-DVE reciprocal needs
            # an SBUF source anyway).
            for h in (0, 1):
                den = small.tile([1, 512], F32, tag="den", name=f"den_{h}_{j}")
                nc.vector.tensor_copy(den[:], av[h][D:VW, :])
                recip = small.tile([1, 512], F32, tag="recip",
                                   name=f"recip_{h}_{j}")
                # custom-DVE op: SBUF source at partition 0 only
                nc.vector.reciprocal_approx_fast(recip[:], den[:])
                rb = small.tile([64, 512], F32, tag="rb", name=f"rb_{h}_{j}")
                nc.gpsimd.partition_broadcast(rb[:], recip[:])
                if h == 0:
                    dst = ot_ab[0:64, js]
                else:
                    dst = small.tile([64, 512], BF16, tag="otb",
                                     name=f"otb_{j}")
                nc.vector.tensor_mul(dst[:], av[h][0:D, :], rb[:])
                if h == 1:
                    nc.sync.dma_start(ot_ab[64:128, js], dst[:])

            # ---------------- group C (tiles paired across row strips) ----
            force_flush(("c", j))
            avc = psum_av.tile([VW, 512], F32, tag="av", name=f"av_2_{j}")
            started_c = [False]
            prev_c = []

            def flush_av_c(last=False):
                for (i, pt_ap, c0) in prev_c:
                    nc.tensor.matmul(
                        avc[:, c0:512], v_ap(2, i), pt_ap,
                        start=(not started_c[0]), stop=last,
                        skip_group_check=True,
                    )
                    started_c[0] = True
                prev_c.clear()

            def s_c(st_half, i, qs, strip):
                it = slice(i * 128, (i + 1) * 128)
                if strip == 0:
                    nc.tensor.matmul(st_half, kt2[0:64, it], qkt2[0:64, qs],
                                     start=True, stop=True,
                                     skip_group_check=True)
                else:
                    nc.tensor.matmul(st_half, qkt2[64:128, it], kt2[64:128, qs],
                                     start=True, stop=True,
                                     skip_group_check=True)

            # full tiles, paired (i0, i0+1) on strips 0/1
            for i0 in range(0, 4 * j, 2):
                st = psum_s.tile([128, 1024], F32, tag="s",
                                 name=f"sc_{j}_{i0}")
                s_c(st[:, 0:512], i0, js, 0)
                s_c(st[:, 512:1024], i0 + 1, js, 1)
                pt = ppool.tile([128, 1024], BF16, tag="pt",
                                name=f"ptc_{j}_{i0}")
                nc.scalar.activation(pt[:], st[:], AF.Exp, scale=SCALE)
                pump(2)
                flush_av_c()
                prev_c.extend([(i0, pt[:, 0:512], 0),
                               (i0 + 1, pt[:, 512:1024], 0)])
                maybe_inject()

            # diagonal tiles, paired (0,1) and (2,3)
            for op in range(2):
                oia, oib = 2 * op, 2 * op + 1
                ia, ib = 4 * j + oia, 4 * j + oib
                c0a, c0b = 128 * oia, 128 * oib
                qsa = slice(j * 512 + c0a, (j + 1) * 512)
                qsb = slice(j * 512 + c0b, (j + 1) * 512)
                st = psum_s.tile([128, 1024], F32, tag="s",
                                 name=f"scd_{j}_{op}")
                s_c(st[:, c0a:512], ia, qsa, 0)
                s_c(st[:, 512 + c0b:1024], ib, qsb, 1)
                pt = ppool.tile([128, 1024], BF16, tag="pt",
                                name=f"ptcd_{j}_{op}")
                nc.scalar.activation(pt[:, c0a:512], st[:, c0a:512],
                                     AF.Exp, scale=SCALE)
                nc.scalar.activation(pt[:, 512 + c0b:1024],
                                     st[:, 512 + c0b:1024],
                                     AF.Exp, scale=SCALE)
                nc.vector.tensor_mul(
                    pt[:, c0a:c0a + 128], pt[:, c0a:c0a + 128],
                    masks[:, 0:128])
                nc.vector.tensor_mul(
                    pt[:, 512 + c0b:512 + c0b + 128],
                    pt[:, 512 + c0b:512 + c0b + 128],
                    masks[:, 0:128])
                pump(3)
                flush_av_c()
                prev_c.extend([(ia, pt[:, c0a:512], c0a),
                               (ib, pt[:, 512 + c0b:1024], c0b)])
                maybe_inject()
            flush_av_c(last=True)

            # normalize C
            den = small.tile([1, 512], F32, tag="den", name=f"den_2_{j}")
            nc.vector.tensor_copy(den[:], avc[D:VW, :])
            recip = small.tile([1, 512], F32, tag="recip", name=f"recip_2_{j}")
            nc.vector.reciprocal_approx_fast(recip[:], den[:])
            rb = small.tile([64, 512], F32, tag="rb", name=f"rb_2_{j}")
            nc.gpsimd.partition_broadcast(rb[:], recip[:])
            nc.vector.tensor_mul(ot_c[0:64, js], avc[0:D, :], rb[:])
            nc.sync.dma_start(ot_c[64:128, js], ot_c[0:64, js])

        # epilogue: drain the queue, then the last chunk's out-projection with
        # all three jt-pairs on distinct psum banks (pr pool + freed s pool)
        # so nothing WAR-serializes; copies and output DMAs go last.
        while inj_q:
            inj_q.pop(0)[1]()
        j3 = NJ - 1
        js3 = slice(j3 * 512, (j3 + 1) * 512)
        pss = [(psum_pr.tile([128, 512], F32, tag="proj", name="ep0")[:],
                psum_pr.tile([128, 512], F32, tag="proj", name="ep1")[:])]
        for n in range(2):
            st = psum_s.tile([128, 1024], F32, tag="s", name=f"eps{n}")
            pss.append((st[:, 0:512], st[:, 512:1024]))
        for p in range(3):
            a0, a1 = pss[p]
            jt0, jt1 = 2 * p, 2 * p + 1
            nc.tensor.matmul(a0, w2ab[:, jt0 * 128:(jt0 + 1) * 128],
                             ot_ab[:, js3], start=True, stop=False,
                             skip_group_check=True)
            nc.tensor.matmul(a1, w2ab[:, jt1 * 128:(jt1 + 1) * 128],
                             ot_ab[:, js3], start=True, stop=False,
                             skip_group_check=True)
        for p in range(3):
            a0, a1 = pss[p]
            jt0, jt1 = 2 * p, 2 * p + 1
            nc.tensor.matmul(a0, w2c[0:64, jt0 * 128:(jt0 + 1) * 128],
                             ot_c[0:64, js3], start=False, stop=True,
                             skip_group_check=True)
            nc.tensor.matmul(a1, w2c[64:128, jt1 * 128:(jt1 + 1) * 128],
                             ot_c[64:128, js3], start=False, stop=True,
                             skip_group_check=True)
        for p in range(3):
            a0, a1 = pss[p]
            for jt, a in ((2 * p, a0), (2 * p + 1, a1)):
                ob = small.tile([128, 512], F16, tag="ob", name=f"eob_{jt}")
                nc.vector.tensor_copy(ob[:], a)
                nc.sync.dma_start(
                    outp_d[:, jt * T + j3 * 512: jt * T + (j3 + 1) * 512],
                    ob[:])

    nc.compile()
    return nc


def _prep_in_maps(x, qkv_w, qkv_b, out_w):
    bf = ml_dtypes.bfloat16
    in_maps = []

    # causal edge mask: within a diagonal block, keep when local col >= row;
    # identical for every diagonal offset. Doubled for merged A/B steps.
    p = np.arange(128)[:, None]
    f = np.arange(128)[None, :]
    m = (f >= p)
    masks_s = np.ascontiguousarray(
        np.concatenate([m, m], axis=1)).astype(bf)  # [128, 256]

    for c in range(NCORES):
        b = c // 4
        h0 = (c % 4) * HPC
        hs = [h0, h0 + 1, h0 + 2]

        xT = np.ascontiguousarray(x[b].T.astype(np.float32))  # [768, 2048]
        # j-major: [128, j, kc, 512]
        xt_s = xT.reshape(CC, 128, NJ, 512).transpose(1, 2, 0, 3) \
            .reshape(128, NJ * CC * 512)

        qr = lambda h: qkv_w[h * D:(h + 1) * D]
        kr = lambda h: qkv_w[C + h * D: C + (h + 1) * D]
        vr = lambda h: qkv_w[2 * C + h * D: 2 * C + (h + 1) * D]
        qb = lambda h: qkv_b[h * D:(h + 1) * D]
        kb = lambda h: qkv_b[C + h * D: C + (h + 1) * D]

        wqk = np.concatenate([qr(hs[0]), qr(hs[1]), kr(hs[0]), kr(hs[1]),
                              qr(hs[2]), kr(hs[2])], axis=0)  # [384, 768]
        # jt-major: [128, jt, kc, 128]
        wqk_s = np.ascontiguousarray(wqk.T).reshape(CC, 128, 3, 128) \
            .transpose(1, 2, 0, 3).reshape(128, 3 * CC * 128)
        wv_ = np.concatenate([vr(h) for h in hs], axis=0)      # [192, 768]
        wv_s = np.ascontiguousarray(wv_.T).reshape(CC, 128, 192) \
            .transpose(1, 0, 2).reshape(128, CC * 192)

        bqk = np.concatenate([qb(hs[0]), qb(hs[1]), kb(hs[0]), kb(hs[1]),
                              qb(hs[2]), kb(hs[2])])
        bqk_s = np.ascontiguousarray(bqk.reshape(3, 128).T).astype(np.float32)

        ch_ab = np.r_[hs[0] * D:(hs[0] + 1) * D, hs[1] * D:(hs[1] + 1) * D]
        ch_c = np.r_[hs[2] * D:(hs[2] + 1) * D]
        w2ab_s = np.ascontiguousarray(out_w[:, ch_ab].T)  # [128, 768]
        w2c_s = np.ascontiguousarray(out_w[:, ch_c].T)    # [64, 768]

        in_maps.append({
            "xt_s": np.ascontiguousarray(xt_s).astype(bf),
            "wqk_s": np.ascontiguousarray(wqk_s).astype(bf),
            "wv_s": np.ascontiguousarray(wv_s).astype(bf),
            "bqk_s": bqk_s,
            "w2ab_s": w2ab_s.astype(bf),
            "w2c_s": w2c_s.astype(bf),
            "masks_s": masks_s,
        })
    return in_maps


def _assemble(results, qkv_b, out_w, out_b):
    out = np.zeros((B, T, C), dtype=np.float32)
    for c in range(NCORES):
        b = c // 4
        outp = results[c]["outp"].astype(np.float32)  # [128, CC*T] f16
        outT = outp.reshape(128, CC, T).transpose(1, 0, 2).reshape(C, T)
        out[b] += outT.T
    # v-bias contribution (softmax rows sum to 1) + output bias
    const = out_w.astype(np.float32) @ qkv_b[2 * C:].astype(np.float32) \
        + out_b.astype(np.float32)
    out += const[None, None, :]
    return out


def run(x, qkv_w, qkv_b, out_w, out_b, trace=False, tmpdir=None):
    if "nc" not in _cache:
        _cache["nc"] = _build_program()
    nc = _cache["nc"]
    x = np.asarray(x, dtype=np.float32)
    qkv_w = np.asarray(qkv_w, dtype=np.float32)
    qkv_b = np.asarray(qkv_b, dtype=np.float32)
    out_w = np.asarray(out_w, dtype=np.float32)
    out_b = np.asarray(out_b, dtype=np.float32)
    in_maps = _prep_in_maps(x, qkv_w, qkv_b, out_w)
    res = run_bass_kernel_spmd(nc, in_maps, list(range(NCORES)), trace=trace,
                               tmpdir=tmpdir)
    out = _assemble(res.results, qkv_b, out_w, out_b)
    return out, res


def kernel(x, qkv_w, qkv_b, out_w, out_b):
    out, _ = run(x, qkv_w, qkv_b, out_w, out_b, trace=False)
    return out


# revision 46
# speedup vs baseline: 1.0032x; 1.0032x over previous
"""Causal self-attention (B=2, T=2048, C=768, H=12) on 8 TRN2 NeuronCores.

Sharding: data-parallel over B (cores 0-3 -> b=0, cores 4-7 -> b=1), tensor
parallel over heads (3 heads per core). Each core computes q/k/v projections
for its 3 heads, causal attention, and a partial output projection; the host
sums the 4 partials per batch element and adds the output/v biases.

Attention is computed transposed: S^T[tk, tq] = K Q^T so that the softmax
denominator comes out of the ones-augmented AV matmul (V | 1) as row 64 of
the [65, 512] PSUM accumulator; exp runs on the scalar engine straight out
of PSUM with 1/sqrt(d) folded into the activation scale.

PE sub-array concurrency (measured 1.95x for K=64 pairs on disjoint row
strips): the S matmuls for heads A and B occupy PE row strips 0-63 / 64-127
and write the two banks of one [128, 1024] PSUM tile, so the pair executes
concurrently and one merged exp covers both heads. Head C's S matmuls are
paired tile-on-tile the same way using a second copy of qC/kC on the other
partition strip. The K=64 half of the out-projection is likewise paired
across jt using duplicated w2c / ot_c rows.

The attention phase is ACT(exp)-paced, so the q/k/v projection matmuls for
chunk j+1 and the out-projection for chunk j-1 are injected between the
attention tile-steps of chunk j to keep PE busy while ACT drains.

The v bias never touches the device: softmax rows sum to 1, so its
contribution is the constant vector out_w @ qkv_b[2C:], added on the host.
"""

import numpy as np
import ml_dtypes
from contextlib import ExitStack

import concourse.bass as bass
import concourse.tile as tile
from concourse import bacc, mybir
from concourse.bass_utils import run_bass_kernel_spmd

BF16 = mybir.dt.bfloat16
F16 = mybir.dt.float16
F32 = mybir.dt.float32
AF = mybir.ActivationFunctionType

B, T, C, H, D = 2, 2048, 768, 12, 64
HPC = 3          # heads per core
NCORES = 8
CC = C // 128    # 6 contraction chunks
NT = T // 128    # 16 t tiles
NJ = T // 512    # 4 tq chunks
VW = D + 1       # 65: v columns + ones column
SCALE = float(D) ** -0.5

_cache = {}


def _build_program():
    nc = bacc.Bacc("TRN2", target_bir_lowering=False, debug=False,
                   enable_asserts=False, num_devices=NCORES)

    # xt: j-major [j, kc, 512]; wqk: jt-major [jt, kc, 128]
    xt_d = nc.dram_tensor("xt_s", [128, NJ * CC * 512], BF16,
                          kind="ExternalInput").ap()
    wqk_d = nc.dram_tensor("wqk_s", [128, 3 * CC * 128], BF16,
                           kind="ExternalInput").ap()
    wv_d = nc.dram_tensor("wv_s", [128, CC * 192], BF16,
                          kind="ExternalInput").ap()
    bqk_d = nc.dram_tensor("bqk_s", [128, 3], F32, kind="ExternalInput").ap()
    w2ab_d = nc.dram_tensor("w2ab_s", [128, C], BF16, kind="ExternalInput").ap()
    w2c_d = nc.dram_tensor("w2c_s", [64, C], BF16, kind="ExternalInput").ap()
    # causal edge mask [128, 128] (same for every diag offset), doubled for
    # the merged A/B diag steps: [mask | mask]
    masks_d = nc.dram_tensor("masks_s", [128, 256], BF16,
                             kind="ExternalInput").ap()
    outp_d = nc.dram_tensor("outp", [128, CC * T], F16, kind="ExternalOutput").ap()

    with tile.TileContext(nc) as tc, ExitStack() as ctx:
        const = ctx.enter_context(tc.tile_pool(name="const", bufs=1))
        big = ctx.enter_context(tc.tile_pool(name="big", bufs=1))
        # PSUM budget (8 banks): s 2x[128,1024] = 4, av 2x[65,512] = 2,
        # pr 2x[128,512] = 2
        psum_s = ctx.enter_context(tc.tile_pool(name="psum_s", bufs=2,
                                                space="PSUM"))
        psum_av = ctx.enter_context(tc.tile_pool(name="psum_av", bufs=2,
                                                 space="PSUM"))
        psum_pr = ctx.enter_context(tc.tile_pool(name="psum_pr", bufs=2,
                                                 space="PSUM"))
        ppool = ctx.enter_context(tc.tile_pool(name="ppool", bufs=4))
        small = ctx.enter_context(tc.tile_pool(name="small", bufs=6))

        # warm the ACT exp table while DMAs are in flight
        warm = small.tile([1, 16], F32, tag="warm")
        nc.vector.memset(warm[:], 0.0)
        warm2 = small.tile([1, 16], F32, tag="warm")
        nc.scalar.activation(warm2[:], warm[:], AF.Exp)

        # ---- load constants/inputs (order matters: earliest consumers first)
        wqk = const.tile([128, 3 * CC * 128], BF16)
        nc.sync.dma_start(wqk[:, 0:128], wqk_d[:, 0:128])       # jt=0 kc=0
        nc.sync.dma_start(wqk[:, 128:768], wqk_d[:, 128:768])   # jt=0 rest
        xt = const.tile([128, NJ * CC * 512], BF16)
        for kc in range(CC):                                    # xt chunk j=0
            sl = slice(kc * 512, (kc + 1) * 512)
            nc.sync.dma_start(xt[:, sl], xt_d[:, sl])
        nc.sync.dma_start(wqk[:, 768:1536], wqk_d[:, 768:1536])   # jt=1
        nc.sync.dma_start(wqk[:, 1536:2304], wqk_d[:, 1536:2304])  # jt=2
        bqk = const.tile([128, 3], F32)
        nc.sync.dma_start(bqk[:], bqk_d[:])
        wv = const.tile([128, CC * 192], BF16)
        nc.sync.dma_start(wv[:], wv_d[:])
        nc.sync.dma_start(xt[:, 3072:6144], xt_d[:, 3072:6144])   # j=1
        masks = const.tile([128, 256], BF16)
        nc.sync.dma_start(masks[:], masks_d[:])
        w2ab = const.tile([128, C], BF16)
        nc.sync.dma_start(w2ab[:], w2ab_d[:])
        w2c = const.tile([128, C], BF16)   # same 64 rows on both strips
        nc.sync.dma_start(w2c[0:64, :], w2c_d[:])
        nc.sync.dma_start(w2c[64:128, :], w2c_d[:])
        nc.sync.dma_start(xt[:, 6144:9216], xt_d[:, 6144:9216])   # j=2
        nc.sync.dma_start(xt[:, 9216:12288], xt_d[:, 9216:12288])  # j=3

        # ---- persistent intermediates
        qt1 = big.tile([128, T], BF16)   # qA (p 0-63) | qB (p 64-127), [d, t]
        kt1 = big.tile([128, T], BF16)   # kA | kB
        qkt2 = big.tile([128, T], BF16)  # qC | kC
        kt2 = big.tile([128, T], BF16)   # kC (p 0-63) | qC (p 64-127)
        vbuf = big.tile([128, NT * HPC * VW], BF16)  # per t-chunk: [vA 1|vB 1|vC 1]
        ot_ab = big.tile([128, T], BF16)  # O.T heads A,B (out-proj rhs chunk 0)
        ot_c = big.tile([128, T], BF16)   # O.T head C on both strips

        nc.gpsimd.memset(vbuf[:], 1.0)

        def v_ap(h, i):
            off = i * HPC * VW + h * VW
            return vbuf[:, off:off + VW]

        qk_dest = [qt1, kt1, qkt2]

        def xt_ap(kc, j):
            return xt[:, j * 3072 + kc * 512: j * 3072 + (kc + 1) * 512]

        def qk_parts(jt, j):
            """Injectable closures: 3x two accumulating matmuls + bias add."""
            state = {}

            def mk(kc0):
                def f():
                    if "ps" not in state:
                        state["ps"] = psum_pr.tile([128, 512], F32, tag="proj",
                                                   name=f"qk_{jt}_{j}")
                    for kc in (kc0, kc0 + 1):
                        nc.tensor.matmul(
                            state["ps"][:],
                            wqk[:, jt * 768 + kc * 128: jt * 768 + (kc + 1) * 128],
                            xt_ap(kc, j),
                            start=(kc == 0), stop=(kc == CC - 1),
                        )
                return f

            def fin():
                nc.vector.tensor_scalar_add(
                    qk_dest[jt][:, j * 512:(j + 1) * 512], state["ps"][:],
                    bqk[:, jt:jt + 1])

            return [mk(0), mk(2), mk(4), fin]

        def emit_qk_group(jt, j):
            for f in qk_parts(jt, j):
                f()

        def emit_shift(j):
            js = slice(j * 512, (j + 1) * 512)
            nc.sync.dma_start(kt2[0:64, js], qkt2[64:128, js])
            nc.sync.dma_start(kt2[64:128, js], qkt2[0:64, js])

        def v_parts(ti):
            state = {}
            jv, rv = ti // 4, ti % 4

            def mk(kc0):
                def f():
                    if "ps" not in state:
                        state["ps"] = psum_pr.tile([128, 192], F32, tag="proj",
                                                   name=f"v_{ti}")
                    for kc in (kc0, kc0 + 1):
                        nc.tensor.matmul(
                            state["ps"][:],
                            xt[:, jv * 3072 + kc * 512 + rv * 128:
                               jv * 3072 + kc * 512 + (rv + 1) * 128],
                            wv[:, kc * 192:(kc + 1) * 192],
                            start=(kc == 0), stop=(kc == CC - 1),
                        )
                return f

            def fin():
                dst = vbuf[:, ti * HPC * VW:(ti + 1) * HPC * VW]
                dst = dst.rearrange("p (h x) -> p h x", h=HPC)[:, :, 0:D]
                nc.vector.tensor_copy(
                    dst, state["ps"][:].rearrange("p (h x) -> p h x", h=HPC))

            return [mk(0), mk(2), mk(4), fin]

        def emit_v_group(ti):
            for f in v_parts(ti):
                f()

        def op_parts(p, j):
            # jt pair (2p, 2p+1); the K=64 w2c matmuls run concurrently on
            # opposite PE row strips.
            jt0, jt1 = 2 * p, 2 * p + 1
            js = slice(j * 512, (j + 1) * 512)
            state = {}

            def mm1():
                ps0 = psum_pr.tile([128, 512], F32, tag="proj",
                                   name=f"op0_{p}_{j}")
                ps1 = psum_pr.tile([128, 512], F32, tag="proj",
                                   name=f"op1_{p}_{j}")
                state["ps"] = (ps0, ps1)
                nc.tensor.matmul(ps0[:], w2ab[:, jt0 * 128:(jt0 + 1) * 128],
                                 ot_ab[:, js], start=True, stop=False,
                                 skip_group_check=True)
                nc.tensor.matmul(ps1[:], w2ab[:, jt1 * 128:(jt1 + 1) * 128],
                                 ot_ab[:, js], start=True, stop=False,
                                 skip_group_check=True)

            def mm2():
                ps0, ps1 = state["ps"]
                nc.tensor.matmul(ps0[:], w2c[0:64, jt0 * 128:(jt0 + 1) * 128],
                                 ot_c[0:64, js], start=False, stop=True,
                                 skip_group_check=True)
                nc.tensor.matmul(ps1[:], w2c[64:128, jt1 * 128:(jt1 + 1) * 128],
                                 ot_c[64:128, js], start=False, stop=True,
                                 skip_group_check=True)

            def fin():
                for jt, ps in zip((jt0, jt1), state["ps"]):
                    ob = small.tile([128, 512], F16, tag="ob",
                                    name=f"ob_{jt}_{j}")
                    nc.vector.tensor_copy(ob[:], ps[:])
                    nc.sync.dma_start(
                        outp_d[:, jt * T + j * 512: jt * T + (j + 1) * 512],
                        ob[:])

            return [mm1, mm2, fin]

        def emit_outproj_pair(p, j):
            for f in op_parts(p, j):
                f()

        # warm-up matmuls: PE activity during the input DMA wait so the HAM
        # clock-gate ramps before real work starts
        wz = const.tile([128, 512], BF16, name="wz")
        nc.gpsimd.memset(wz[:], 0.0)

        for i in range(8):
            wps = psum_pr.tile([128, 512], F32, tag="proj", name=f"wps{i}")
            nc.tensor.matmul(wps[:], wz[:, 0:128], wz[:], start=True, stop=True)

        # prologue: only the projections gating chunk 0's first S-pair;
        # the rest goes on the deadline queue below
        emit_qk_group(0, 0)
        emit_qk_group(1, 0)

        # Global injection queue: ~2-matmul closures popped ONE at a time at
        # pump points inside attention tile-steps, so the next S-pair is never
        # buried behind a convoy of projection matmuls (which starves ACT).
        # Items carry a deadline; force_flush runs everything due by a phase.
        inj_q = []   # (deadline | None, closure)
        inj_q.extend((("vd", 0), f) for ti in range(4) for f in v_parts(ti))
        inj_q.extend((("c", 0), f) for f in qk_parts(2, 0))
        inj_q.append((("c", 0), lambda: emit_shift(0)))

        def pump(n=1):
            for _ in range(min(n, len(inj_q))):
                inj_q.pop(0)[1]()

        def force_flush(dl):
            keep = []
            for item in inj_q:
                if item[0] == dl:
                    item[1]()
                else:
                    keep.append(item)
            inj_q[:] = keep

        for j in range(NJ):
            js = slice(j * 512, (j + 1) * 512)

            if j + 1 < NJ:
                for jt in (0, 1):
                    inj_q.extend((("ab", j + 1), f) for f in qk_parts(jt, j + 1))
                inj_q.extend((("c", j + 1), f) for f in qk_parts(2, j + 1))
                inj_q.append((("c", j + 1), lambda jn=j + 1: emit_shift(jn)))
                for ti in range(4 * (j + 1), 4 * (j + 1) + 4):
                    inj_q.extend((("vd", j + 1), f) for f in v_parts(ti))
            if j - 1 >= 0:
                for p in range(3):
                    inj_q.extend((None, f) for f in op_parts(p, j - 1))

            # everything chunk j's AB group depends on must be emitted already
            force_flush(("ab", j))

            def maybe_inject():
                pump()

            # ---------------- group A,B (row strips 0-63 / 64-127) --------
            av = {h: psum_av.tile([VW, 512], F32, tag="av",
                                  name=f"av_{h}_{j}") for h in (0, 1)}
            started = {0: False, 1: False}
            prev = []   # (h, i, pt_ap, col0) from previous tile-step

            def flush_av(last=False):
                for (h, i, pt_ap, c0) in prev:
                    nc.tensor.matmul(
                        av[h][:, c0:512], v_ap(h, i), pt_ap,
                        start=(not started[h]), stop=last,
                        skip_group_check=True,
                    )
                    started[h] = True
                prev.clear()

            # full tiles (i < 4j): S_A and S_B write the two banks of one
            # [128, 1024] PSUM tile and execute concurrently; one merged exp.
            for i0 in range(4 * j):
                it = slice(i0 * 128, (i0 + 1) * 128)
                st = psum_s.tile([128, 1024], F32, tag="s",
                                 name=f"sab_{j}_{i0}")
                nc.tensor.matmul(st[:, 0:512], kt1[0:64, it], qt1[0:64, js],
                                 start=True, stop=True, skip_group_check=True)
                nc.tensor.matmul(st[:, 512:1024], kt1[64:128, it],
                                 qt1[64:128, js],
                                 start=True, stop=True, skip_group_check=True)
                pt = ppool.tile([128, 1024], BF16, tag="pt",
                                name=f"ptab_{j}_{i0}")
                nc.scalar.activation(pt[:], st[:], AF.Exp, scale=SCALE)
                pump()
                flush_av()
                prev.extend([(0, i0, pt[:, 0:512], 0),
                             (1, i0, pt[:, 512:1024], 0)])
                maybe_inject()

            # diagonal tiles i = 4j+oi, restricted to cols >= 128*oi
            force_flush(("vd", j))
            for oi in range(4):
                i = 4 * j + oi
                it = slice(i * 128, (i + 1) * 128)
                c0 = 128 * oi
                w = 512 - c0
                qs = slice(j * 512 + c0, (j + 1) * 512)
                st = psum_s.tile([128, 1024], F32, tag="s",
                                 name=f"sabd_{j}_{oi}")
                nc.tensor.matmul(st[:, c0:512], kt1[0:64, it], qt1[0:64, qs],
                                 start=True, stop=True, skip_group_check=True)
                nc.tensor.matmul(st[:, 512 + c0:1024], kt1[64:128, it],
                                 qt1[64:128, qs],
                                 start=True, stop=True, skip_group_check=True)
                pt = ppool.tile([128, 1024], BF16, tag="pt",
                                name=f"ptabd_{j}_{oi}")
                st3 = st[:].rearrange("p (two q) -> p two q", two=2)[:, :, c0:512]
                pt3 = pt[:].rearrange("p (two q) -> p two q", two=2)[:, :, c0:512]
                nc.scalar.activation(pt3, st3, AF.Exp, scale=SCALE)
                # the invalid triangle only spans cols [c0, c0+128)
                ptm = pt[:].rearrange("p (two q) -> p two q", two=2) \
                    [:, :, c0:c0 + 128]
                m3 = masks[:].rearrange("p (two q) -> p two q", two=2)
                nc.vector.tensor_mul(ptm, ptm, m3)
                pump(2)
                flush_av()
                prev.extend([(0, i, pt[:, c0:512], c0),
                             (1, i, pt[:, 512 + c0:1024], c0)])
                maybe_inject()
            flush_av(last=True)

            # normalize A, B: O.T[d, tq] = av[0:64] / av[64].  The whole av
            # accumulator is copied to SBUF first so the PSUM bank frees in
            # one op instead of being held through the normalize chain; the
            # copy doubles as the den staging (custom-DVE reciprocal needs
            # an SBUF source anyway).
            for h in (0, 1):
                den = small.tile([1, 512], F32, tag="den", name=f"den_{h}_{j}")
                nc.vector.tensor_copy(den[:], av[h][D:VW, :])
                recip = small.tile([1, 512], F32, tag="recip",
                                   name=f"recip_{h}_{j}")
                # custom-DVE op: SBUF source at partition 0 only
                nc.vector.reciprocal_approx_fast(recip[:], den[:])
                rb = small.tile([64, 512], F32, tag="rb", name=f"rb_{h}_{j}")
                nc.gpsimd.partition_broadcast(rb[:], recip[:])
                if h == 0:
                    dst = ot_ab[0:64, js]
                else:
                    dst = small.tile([64, 512], BF16, tag="otb",
                                     name=f"otb_{j}")
                nc.vector.tensor_mul(dst[:], av[h][0:D, :], rb[:])
                if h == 1:
                    nc.sync.dma_start(ot_ab[64:128, js], dst[:])

            # ---------------- group C (tiles paired across row strips) ----
            force_flush(("c", j))
            avc = psum_av.tile([VW, 512], F32, tag="av", name=f"av_2_{j}")
            started_c = [False]
            prev_c = []

            def flush_av_c(last=False):
                for (i, pt_ap, c0) in prev_c:
                    nc.tensor.matmul(
                        avc[:, c0:512], v_ap(2, i), pt_ap,
                        start=(not started_c[0]), stop=last,
                        skip_group_check=True,
                    )
                    started_c[0] = True
                prev_c.clear()

            def s_c(st_half, i, qs, strip):
                it = slice(i * 128, (i + 1) * 128)
                if strip == 0:
                    nc.tensor.matmul(st_half, kt2[0:64, it], qkt2[0:64, qs],
                                     start=True, stop=True,
                                     skip_group_check=True)
                else:
                    nc.tensor.matmul(st_half, qkt2[64:128, it], kt2[64:128, qs],
                                     start=True, stop=True,
                                     skip_group_check=True)

            # full tiles, paired (i0, i0+1) on strips 0/1
            for i0 in range(0, 4 * j, 2):
                st = psum_s.tile([128, 1024], F32, tag="s",
                                 name=f"sc_{j}_{i0}")
                s_c(st[:, 0:512], i0, js, 0)
                s_c(st[:, 512:1024], i0 + 1, js, 1)
                pt = ppool.tile([128, 1024], BF16, tag="pt",
                                name=f"ptc_{j}_{i0}")
                nc.scalar.activation(pt[:], st[:], AF.Exp, scale=SCALE)
                pump(2)
                flush_av_c()
                prev_c.extend([(i0, pt[:, 0:512], 0),
                               (i0 + 1, pt[:, 512:1024], 0)])
                maybe_inject()

            # diagonal tiles, paired (0,1) and (2,3)
            for op in range(2):
                oia, oib = 2 * op, 2 * op + 1
                ia, ib = 4 * j + oia, 4 * j + oib
                c0a, c0b = 128 * oia, 128 * oib
                qsa = slice(j * 512 + c0a, (j + 1) * 512)
                qsb = slice(j * 512 + c0b, (j + 1) * 512)
                st = psum_s.tile([128, 1024], F32, tag="s",
                                 name=f"scd_{j}_{op}")
                s_c(st[:, c0a:512], ia, qsa, 0)
                s_c(st[:, 512 + c0b:1024], ib, qsb, 1)
                pt = ppool.tile([128, 1024], BF16, tag="pt",
                                name=f"ptcd_{j}_{op}")
                nc.scalar.activation(pt[:, c0a:512], st[:, c0a:512],
                                     AF.Exp, scale=SCALE)
                nc.scalar.activation(pt[:, 512 + c0b:1024],
                                     st[:, 512 + c0b:1024],
                                     AF.Exp, scale=SCALE)
                nc.vector.tensor_mul(
                    pt[:, c0a:c0a + 128], pt[:, c0a:c0a + 128],
                    masks[:, 0:128])
                nc.vector.tensor_mul(
                    pt[:, 512 + c0b:512 + c0b + 128],
                    pt[:, 512 + c0b:512 + c0b + 128],
                    masks[:, 0:128])
                pump(3)
                flush_av_c()
                prev_c.extend([(ia, pt[:, c0a:512], c0a),
                               (ib, pt[:, 512 + c0b:1024], c0b)])
                maybe_inject()
            flush_av_c(last=True)

            # normalize C
            den = small.tile([1, 512], F32, tag="den", name=f"den_2_{j}")
            nc.vector.tensor_copy(den[:], avc[D:VW, :])
            recip = small.tile([1, 512], F32, tag="recip", name=f"recip_2_{j}")
            nc.vector.reciprocal_approx_fast(recip[:], den[:])
            rb = small.tile([64, 512], F32, tag="rb", name=f"rb_2_{j}")
            nc.gpsimd.partition_broadcast(rb[:], recip[:])
            nc.vector.tensor_mul(ot_c[0:64, js], avc[0:D, :], rb[:])
            if j < NJ - 1:
                # last chunk's w2c matmuls run unpaired on strip 0, so the
                # tail needs no ot_c shift DMA
                nc.sync.dma_start(ot_c[64:128, js], ot_c[0:64, js])

        # epilogue: drain the queue, then the last chunk's out-projection with
        # all three jt-pairs on distinct psum banks (pr pool + freed s pool)
        # so nothing WAR-serializes; copies and output DMAs go last.
        while inj_q:
            inj_q.pop(0)[1]()
        j3 = NJ - 1
        js3 = slice(j3 * 512, (j3 + 1) * 512)
        pss = [(psum_pr.tile([128, 512], F32, tag="proj", name="ep0")[:],
                psum_pr.tile([128, 512], F32, tag="proj", name="ep1")[:])]
        for n in range(2):
            st = psum_s.tile([128, 1024], F32, tag="s", name=f"eps{n}")
            pss.append((st[:, 0:512], st[:, 512:1024]))
        for p in range(3):
            a0, a1 = pss[p]
            jt0, jt1 = 2 * p, 2 * p + 1
            nc.tensor.matmul(a0, w2ab[:, jt0 * 128:(jt0 + 1) * 128],
                             ot_ab[:, js3], start=True, stop=False,
                             skip_group_check=True)
            nc.tensor.matmul(a1, w2ab[:, jt1 * 128:(jt1 + 1) * 128],
                             ot_ab[:, js3], start=True, stop=False,
                             skip_group_check=True)
        for p in range(3):
            a0, a1 = pss[p]
            jt0, jt1 = 2 * p, 2 * p + 1
            nc.tensor.matmul(a0, w2c[0:64, jt0 * 128:(jt0 + 1) * 128],
                             ot_c[0:64, js3], start=False, stop=True,
                             skip_group_check=True)
            nc.tensor.matmul(a1, w2c[0:64, jt1 * 128:(jt1 + 1) * 128],
                             ot_c[0:64, js3], start=False, stop=True,
                             skip_group_check=True)
        for p in range(3):
            a0, a1 = pss[p]
            for jt, a in ((2 * p, a0), (2 * p + 1, a1)):
                ob = small.tile([128, 512], F16, tag="ob", name=f"eob_{jt}")
                nc.vector.tensor_copy(ob[:], a)
                nc.sync.dma_start(
                    outp_d[:, jt * T + j3 * 512: jt * T + (j3 + 1) * 512],
                    ob[:])

    nc.compile()
    return nc


def _prep_in_maps(x, qkv_w, qkv_b, out_w):
    bf = ml_dtypes.bfloat16
    in_maps = []

    # causal edge mask: within a diagonal block, keep when local col >= row;
    # identical for every diagonal offset. Doubled for merged A/B steps.
    p = np.arange(128)[:, None]
    f = np.arange(128)[None, :]
    m = (f >= p)
    masks_s = np.ascontiguousarray(
        np.concatenate([m, m], axis=1)).astype(bf)  # [128, 256]

    for c in range(NCORES):
        b = c // 4
        h0 = (c % 4) * HPC
        hs = [h0, h0 + 1, h0 + 2]

        xT = np.ascontiguousarray(x[b].T.astype(np.float32))  # [768, 2048]
        # j-major: [128, j, kc, 512]
        xt_s = xT.reshape(CC, 128, NJ, 512).transpose(1, 2, 0, 3) \
            .reshape(128, NJ * CC * 512)

        qr = lambda h: qkv_w[h * D:(h + 1) * D]
        kr = lambda h: qkv_w[C + h * D: C + (h + 1) * D]
        vr = lambda h: qkv_w[2 * C + h * D: 2 * C + (h + 1) * D]
        qb = lambda h: qkv_b[h * D:(h + 1) * D]
        kb = lambda h: qkv_b[C + h * D: C + (h + 1) * D]

        wqk = np.concatenate([qr(hs[0]), qr(hs[1]), kr(hs[0]), kr(hs[1]),
                              qr(hs[2]), kr(hs[2])], axis=0)  # [384, 768]
        # jt-major: [128, jt, kc, 128]
        wqk_s = np.ascontiguousarray(wqk.T).reshape(CC, 128, 3, 128) \
            .transpose(1, 2, 0, 3).reshape(128, 3 * CC * 128)
        wv_ = np.concatenate([vr(h) for h in hs], axis=0)      # [192, 768]
        wv_s = np.ascontiguousarray(wv_.T).reshape(CC, 128, 192) \
            .transpose(1, 0, 2).reshape(128, CC * 192)

        bqk = np.concatenate([qb(hs[0]), qb(hs[1]), kb(hs[0]), kb(hs[1]),
                              qb(hs[2]), kb(hs[2])])
        bqk_s = np.ascontiguousarray(bqk.reshape(3, 128).T).astype(np.float32)

        ch_ab = np.r_[hs[0] * D:(hs[0] + 1) * D, hs[1] * D:(hs[1] + 1) * D]
        ch_c = np.r_[hs[2] * D:(hs[2] + 1) * D]
        w2ab_s = np.ascontiguousarray(out_w[:, ch_ab].T)  # [128, 768]
        w2c_s = np.ascontiguousarray(out_w[:, ch_c].T)    # [64, 768]

        in_maps.append({
            "xt_s": np.ascontiguousarray(xt_s).astype(bf),
            "wqk_s": np.ascontiguousarray(wqk_s).astype(bf),
            "wv_s": np.ascontiguousarray(wv_s).astype(bf),
            "bqk_s": bqk_s,
            "w2ab_s": w2ab_s.astype(bf),
            "w2c_s": w2c_s.astype(bf),
            "masks_s": masks_s,
        })
    return in_maps


def _assemble(results, qkv_b, out_w, out_b):
    out = np.zeros((B, T, C), dtype=np.float32)
    for c in range(NCORES):
        b = c // 4
        outp = results[c]["outp"].astype(np.float32)  # [128, CC*T] f16
        outT = outp.reshape(128, CC, T).transpose(1, 0, 2).reshape(C, T)
        out[b] += outT.T
    # v-bias contribution (softmax rows sum to 1) + output bias
    const = out_w.astype(np.float32) @ qkv_b[2 * C:].astype(np.float32) \
        + out_b.astype(np.float32)
    out += const[None, None, :]
    return out


def run(x, qkv_w, qkv_b, out_w, out_b, trace=False, tmpdir=None):
    if "nc" not in _cache:
        _cache["nc"] = _build_program()
    nc = _cache["nc"]
    x = np.asarray(x, dtype=np.float32)
    qkv_w = np.asarray(qkv_w, dtype=np.float32)
    qkv_b = np.asarray(qkv_b, dtype=np.float32)
    out_w = np.asarray(out_w, dtype=np.float32)
    out_b = np.asarray(out_b, dtype=np.float32)
    in_maps = _prep_in_maps(x, qkv_w, qkv_b, out_w)
    res = run_bass_kernel_spmd(nc, in_maps, list(range(NCORES)), trace=trace,
                               tmpdir=tmpdir)
    out = _assemble(res.results, qkv_b, out_w, out_b)
    return out, res


def kernel(x, qkv_w, qkv_b, out_w, out_b):
    out, _ = run(x, qkv_w, qkv_b, out_w, out_b, trace=False)
    return out
